# revision 8
# baseline (speedup 1.0000x reference)
"""Dcls2_1d (dilated conv with learnable row spacings) on 8 trn2 NeuronCores.

Strategy: data-parallel over batch (16 -> 2 images/core). Host constructs the
dense (O, I, 7, 3) scattered kernel (exact port of the reference bilinear
scatter, ~0.7 MFLOP) and pads x; each core runs the conv as an implicit GEMM:
for every output chunk of 512 pixels, 21 taps are accumulated in PSUM via
matmuls contracting over C_in=128 (the partition dim), then the bias add is
fused into the PSUM->SBUF evacuation on the scalar engine.

float32r matmuls stream 1 row/cycle (vs 4 for float32) at ~tf32 precision.
Input DMAs are ordered so the first accumulation group's operands land first
(spread over all 16 DMA queues), and a short burst of dummy matmuls warms the
PE clock (HAM) while the real inputs are still in flight.
"""
import os
import sys
import time

sys.path.insert(0, "/opt/trn_rl_repo")

import ml_dtypes
import numpy as np

import concourse.bass as bass
import concourse.tile as tile
from concourse import bacc, mybir
from concourse import bass_utils

# ---- problem constants (hardcoded per contract) ----
K_H, K_W = 3, 3
LIM = 2            # DIL // 2
KH_EFF = 7         # K_H + 2 * LIM
PAD_H, PAD_W = 3, 1
B, CIN, H, W = 16, 128, 64, 64
COUT = 256
N_CORES = 8
BPC = B // N_CORES                  # images per core
HP, WP = H + 2 * PAD_H, W + 2 * PAD_W   # 70, 66
NPIX = H * W                        # 4096
CHUNK = 512                         # output pixels per PSUM bank
NCHUNK = NPIX // CHUNK              # 8
RPC = CHUNK // W                    # rows per chunk: 8
NTAPS = KH_EFF * K_W                # 21
OH = COUT // 128                    # 2 halves of out channels

DT = os.environ.get("DCLS_DT", "f32r")          # f32r | fp16 | bf16 | f32
ORDER = os.environ.get("DCLS_ORDER", "chunk")    # chunk | tap
WARMUP = int(os.environ.get("DCLS_WARMUP", "24"))
_MM_DT = {"f32r": mybir.dt.float32r, "fp16": mybir.dt.float16,
          "bf16": mybir.dt.bfloat16, "f32": mybir.dt.float32}[DT]
_NP_DT = {"f32r": np.float32, "fp16": np.float16,
          "bf16": ml_dtypes.bfloat16, "f32": np.float32}[DT]

if os.environ.get("DCLS_LDWOPT", "0") == "1":
    _orig_run_command = bass_utils.run_command

    def _patched_run_command(cmd, **kw):
        cmd = ["--enable-ldw-opt=true" if c == "--enable-ldw-opt=false" else c
               for c in cmd]
        return _orig_run_command(cmd, **kw)

    bass_utils.run_command = _patched_run_command

_NC_CACHE = None
_last_in_maps = None  # stashed for test.py's profiled re-run


def _build_kernel_np(weight: np.ndarray, P1: np.ndarray) -> np.ndarray:
    """Exact numpy port of reference.build_kernel (fp32)."""
    weight = weight.astype(np.float32, copy=False)
    kh = np.arange(K_H, dtype=np.float32)[None, None, :, None]
    pos = kh + LIM + np.clip(P1.astype(np.float32, copy=False), -LIM, LIM)
    p0 = np.floor(pos)
    frac = pos - p0
    p0i = p0.astype(np.int32)
    rng = np.arange(KH_EFF, dtype=np.int32)
    oh0 = (p0i[..., None] == rng).astype(np.float32)
    oh1 = ((p0i + 1)[..., None] == rng).astype(np.float32)
    return (
        np.einsum("oihw,oihwk->oikw", weight * (1.0 - frac), oh0)
        + np.einsum("oihw,oihwk->oikw", weight * frac, oh1)
    ).astype(np.float32)


def _splits(total, n):
    """n near-equal [lo, hi) column ranges covering [0, total)."""
    step = (total + n - 1) // n
    return [(j, min(j + step, total)) for j in range(0, total, step)]


def _build_bass():
    mmdt = _MM_DT
    f32 = mybir.dt.float32
    nc = bacc.Bacc("TRN2", target_bir_lowering=False, debug=False,
                   num_devices=N_CORES)
    x_d = nc.dram_tensor("x", [BPC, CIN, HP * WP], mmdt,
                         kind="ExternalInput").ap()
    # oh-major weight layout: [i, (oh, kh, kw, o128)]
    k_d = nc.dram_tensor("k", [CIN, OH * NTAPS * 128], mmdt,
                         kind="ExternalInput").ap()
    b_d = nc.dram_tensor("b", [OH, 128, 1], f32, kind="ExternalInput").ap()
    wz_d = nc.dram_tensor("wz", [128, 16], mmdt,
                          kind="ExternalInput").ap() if WARMUP else None
    o_d = nc.dram_tensor("o", [BPC, OH, 128, NPIX], f32,
                         kind="ExternalOutput").ap()

    HEAD_ROWS = RPC + KH_EFF - 1            # x rows needed by first chunk: 14
    HEAD = HEAD_ROWS * WP                   # 924 cols

    # DMA descriptor issue costs ~0.6us on an engine queue; spread issues
    # over four otherwise-idle engine queues so they go out in parallel.
    _rr = [0]

    def dma(engines, dst, src):
        eng = engines[_rr[0] % len(engines)]
        _rr[0] += 1
        eng.dma_start(dst, src)

    with tile.TileContext(nc) as tc:
        with tc.tile_pool(name="xp", bufs=1) as xp, \
             tc.tile_pool(name="kp", bufs=1) as kp, \
             tc.tile_pool(name="bp", bufs=1) as bp, \
             tc.tile_pool(name="wu", bufs=1) as wu, \
             tc.tile_pool(name="ps", bufs=8, space="PSUM") as ps, \
             tc.tile_pool(name="op", bufs=4) as op:

            kt = kp.tile([CIN, OH * NTAPS * 128], mmdt, tag="k")
            bt = bp.tile([128, OH], f32, tag="bias")
            xts = [xp.tile([CIN, HP * WP], mmdt, tag=f"x{n}", name=f"x{n}")
                   for n in range(BPC)]

            # warmup tile for the PE clock (HAM) ramp: tiny zeros input,
            # DMA'd before everything so dummy matmuls start ~1us in
            wt = None
            if WARMUP:
                wt = wu.tile([128, 16], mmdt, tag="warm")
                nc.sync.dma_start(wt[:], wz_d[:])

            # --- input DMAs, priority-ordered, issued from 4 engines in
            # parallel, spread over the 16 HW queues ---
            ie = [nc.sync, nc.gpsimd, nc.scalar]
            # 1) weights for the first oh half
            for lo, hi in _splits(NTAPS * 128, 12):
                dma(ie, kt[:, lo:hi], k_d[:, lo:hi])
            # 2) first rows of image 0
            for lo, hi in _splits(HEAD, 4):
                dma(ie, xts[0][:, lo:hi], x_d[0][:, lo:hi])
            # 3) rest of image 0
            for lo, hi in _splits(HP * WP - HEAD, 5):
                dma(ie, xts[0][:, HEAD + lo:HEAD + hi],
                    x_d[0][:, HEAD + lo:HEAD + hi])
            # 4) bias, second weight half, remaining images
            for h in range(OH):
                dma(ie, bt[:, h:h + 1], b_d[h])
            for lo, hi in _splits(NTAPS * 128, 8):
                off = NTAPS * 128
                dma(ie, kt[:, off + lo:off + hi], k_d[:, off + lo:off + hi])
            for n in range(1, BPC):
                for lo, hi in _splits(HP * WP, 6):
                    dma(ie, xts[n][:, lo:hi], x_d[n][:, lo:hi])

            # --- HAM warmup: dummy matmuls while inputs stream in ---
            for _ in range(WARMUP):
                pw = ps.tile([16, 16], f32, tag="acc")
                nc.tensor.matmul(pw[:], wt[:, 0:16], wt[:], start=True,
                                 stop=True)

            # --- the conv ---
            def do_group(n, h, c, xv):
                pt = ps.tile([128, CHUNK], f32, tag="acc")
                y0 = c * RPC
                for t, (kh, kw) in enumerate(
                        (kh, kw) for kh in range(KH_EFF)
                        for kw in range(K_W)):
                    rhs = xv[:, y0 + kh:y0 + kh + RPC, kw:kw + W]
                    off = ((h * KH_EFF + kh) * K_W + kw) * 128
                    nc.tensor.matmul(pt[:], kt[:, off:off + 128], rhs,
                                     start=(t == 0), stop=(t == NTAPS - 1))
                ot = op.tile([128, CHUNK], f32, tag="out")
                nc.scalar.activation(ot[:], pt[:],
                                     mybir.ActivationFunctionType.Identity,
                                     bias=bt[:, h:h + 1])
                # split the store so the flush of the last chunk isn't
                # bottlenecked on a single ~22GB/s DMA queue
                last = (n == BPC - 1 and h == OH - 1 and c == NCHUNK - 1)
                oe = [nc.sync, nc.gpsimd]
                for lo, hi in _splits(CHUNK, 4 if last else 2):
                    dma(oe, o_d[n, h][:, c * CHUNK + lo:c * CHUNK + hi],
                        ot[:, lo:hi])

            def do_block_tap_outer(n, h, xv):
                pts = [ps.tile([128, CHUNK], f32, tag="acc",
                               name=f"acc_{n}_{h}_{c}")
                       for c in range(NCHUNK)]
                for t, (kh, kw) in enumerate(
                        (kh, kw) for kh in range(KH_EFF)
                        for kw in range(K_W)):
                    off = ((h * KH_EFF + kh) * K_W + kw) * 128
                    for c in range(NCHUNK):
                        rhs = xv[:, c * RPC + kh:c * RPC + kh + RPC, kw:kw + W]
                        nc.tensor.matmul(pts[c][:], kt[:, off:off + 128], rhs,
                                         start=(t == 0),
                                         stop=(t == NTAPS - 1))
                for c in range(NCHUNK):
                    ot = op.tile([128, CHUNK], f32, tag="out")
                    nc.scalar.activation(ot[:], pts[c][:],
                                         mybir.ActivationFunctionType.Identity,
                                         bias=bt[:, h:h + 1])
                    last = (n == BPC - 1 and h == OH - 1 and c == NCHUNK - 1)
                    oe = [nc.sync, nc.gpsimd]
                    for lo, hi in _splits(CHUNK, 4 if last else 2):
                        dma(oe, o_d[n, h][:, c * CHUNK + lo:c * CHUNK + hi],
                            ot[:, lo:hi])

            for n in range(BPC):
                xv = xts[n][:].rearrange("p (h w) -> p h w", h=HP)
                for h in range(OH):
                    if ORDER == "tap":
                        do_block_tap_outer(n, h, xv)
                    else:
                        for c in range(NCHUNK):
                            do_group(n, h, c, xv)
    t0 = time.time()
    nc.compile()
    print(f"[kernel] bacc compile: {time.time()-t0:.1f}s", file=sys.stderr)
    return nc


def kernel(x: np.ndarray, weight: np.ndarray, bias: np.ndarray,
           P: np.ndarray) -> np.ndarray:
    global _NC_CACHE, _last_in_maps
    x = np.asarray(x, dtype=np.float32)
    weight = np.asarray(weight, dtype=np.float32)
    bias = np.asarray(bias, dtype=np.float32)
    P = np.asarray(P, dtype=np.float32)

    K = _build_kernel_np(weight, P[0])                    # (O, I, 7, 3)
    # device layout: [i, (oh, kh, kw, o128)]
    k_dev = np.ascontiguousarray(
        K.reshape(OH, 128, CIN, KH_EFF, K_W)
        .transpose(2, 0, 3, 4, 1)
        .reshape(CIN, OH * NTAPS * 128)).astype(_NP_DT)

    xpad = np.zeros((B, CIN, HP, WP), np.float32)
    xpad[:, :, PAD_H:PAD_H + H, PAD_W:PAD_W + W] = x
    xpad = xpad.reshape(B, CIN, HP * WP).astype(_NP_DT)

    b_dev = np.ascontiguousarray(bias.reshape(OH, 128, 1))

    if _NC_CACHE is None:
        t0 = time.time()
        _NC_CACHE = _build_bass()
        print(f"[kernel] build+compile total: {time.time()-t0:.1f}s",
              file=sys.stderr)

    wz = np.zeros((128, 16), _NP_DT)
    in_maps = [
        {"x": np.ascontiguousarray(xpad[i * BPC:(i + 1) * BPC]),
         "k": k_dev, "b": b_dev, **({"wz": wz} if WARMUP else {})}
        for i in range(N_CORES)
    ]
    _last_in_maps = in_maps
    t0 = time.time()
    res = bass_utils.run_bass_kernel_spmd(
        _NC_CACHE, in_maps, core_ids=list(range(N_CORES)))
    print(f"[kernel] run (incl. walrus compile on first call): "
          f"{time.time()-t0:.1f}s", file=sys.stderr)
    out = np.concatenate(
        [res.results[i]["o"].reshape(BPC, COUT, H, W)
         for i in range(N_CORES)], axis=0)
    return out


# revision 9
# speedup vs baseline: 1.1781x; 1.1781x over previous
"""Dcls2_1d (dilated conv with learnable row spacings) on 8 trn2 NeuronCores.

Strategy: data-parallel over batch (16 -> 2 images/core). Host constructs the
dense (O, I, 7, 3) scattered kernel (exact port of the reference bilinear
scatter, ~0.7 MFLOP) and pads x; each core runs the conv as an implicit GEMM:
for every output chunk of 512 pixels, 21 taps are accumulated in PSUM via
matmuls contracting over C_in=128 (the partition dim), then the bias add is
fused into the PSUM->SBUF evacuation on the scalar engine.

float32r matmuls stream 1 row/cycle (vs 4 for float32) at ~tf32 precision.
Input DMAs are ordered so the first accumulation group's operands land first
(spread over all 16 DMA queues), and a short burst of dummy matmuls warms the
PE clock (HAM) while the real inputs are still in flight.
"""
import os
import sys
import time

sys.path.insert(0, "/opt/trn_rl_repo")

import ml_dtypes
import numpy as np

import concourse.bass as bass
import concourse.tile as tile
from concourse import bacc, mybir
from concourse import bass_utils

# ---- problem constants (hardcoded per contract) ----
K_H, K_W = 3, 3
LIM = 2            # DIL // 2
KH_EFF = 7         # K_H + 2 * LIM
PAD_H, PAD_W = 3, 1
B, CIN, H, W = 16, 128, 64, 64
COUT = 256
N_CORES = 8
BPC = B // N_CORES                  # images per core
HP, WP = H + 2 * PAD_H, W + 2 * PAD_W   # 70, 66
NPIX = H * W                        # 4096
CHUNK = 512                         # output pixels per PSUM bank
NCHUNK = NPIX // CHUNK              # 8
RPC = CHUNK // W                    # rows per chunk: 8
NTAPS = KH_EFF * K_W                # 21
OH = COUT // 128                    # 2 halves of out channels

DT = os.environ.get("DCLS_DT", "f32r")          # f32r | fp16 | bf16 | f32
ORDER = os.environ.get("DCLS_ORDER", "chunk")    # chunk | tap
WARMUP = int(os.environ.get("DCLS_WARMUP", "24"))
_MM_DT = {"f32r": mybir.dt.float32r, "fp16": mybir.dt.float16,
          "bf16": mybir.dt.bfloat16, "f32": mybir.dt.float32}[DT]
_NP_DT = {"f32r": np.float32, "fp16": np.float16,
          "bf16": ml_dtypes.bfloat16, "f32": np.float32}[DT]

if os.environ.get("DCLS_LDWOPT", "0") == "1":
    _orig_run_command = bass_utils.run_command

    def _patched_run_command(cmd, **kw):
        cmd = ["--enable-ldw-opt=true" if c == "--enable-ldw-opt=false" else c
               for c in cmd]
        return _orig_run_command(cmd, **kw)

    bass_utils.run_command = _patched_run_command

_NC_CACHE = None
_last_in_maps = None  # stashed for test.py's profiled re-run


def _build_kernel_np(weight: np.ndarray, P1: np.ndarray) -> np.ndarray:
    """Exact numpy port of reference.build_kernel (fp32)."""
    weight = weight.astype(np.float32, copy=False)
    kh = np.arange(K_H, dtype=np.float32)[None, None, :, None]
    pos = kh + LIM + np.clip(P1.astype(np.float32, copy=False), -LIM, LIM)
    p0 = np.floor(pos)
    frac = pos - p0
    p0i = p0.astype(np.int32)
    rng = np.arange(KH_EFF, dtype=np.int32)
    oh0 = (p0i[..., None] == rng).astype(np.float32)
    oh1 = ((p0i + 1)[..., None] == rng).astype(np.float32)
    return (
        np.einsum("oihw,oihwk->oikw", weight * (1.0 - frac), oh0)
        + np.einsum("oihw,oihwk->oikw", weight * frac, oh1)
    ).astype(np.float32)


def _splits(total, n):
    """n near-equal [lo, hi) column ranges covering [0, total)."""
    step = (total + n - 1) // n
    return [(j, min(j + step, total)) for j in range(0, total, step)]


def _build_bass():
    mmdt = _MM_DT
    f32 = mybir.dt.float32
    nc = bacc.Bacc("TRN2", target_bir_lowering=False, debug=False,
                   num_devices=N_CORES)
    x_d = nc.dram_tensor("x", [BPC, CIN, HP * WP], mmdt,
                         kind="ExternalInput").ap()
    # oh-major weight layout: [i, (oh, kh, kw, o128)]
    k_d = nc.dram_tensor("k", [CIN, OH * NTAPS * 128], mmdt,
                         kind="ExternalInput").ap()
    b_d = nc.dram_tensor("b", [OH, 128, 1], f32, kind="ExternalInput").ap()
    wz_d = nc.dram_tensor("wz", [128, 16], mmdt,
                          kind="ExternalInput").ap() if WARMUP else None
    o_d = nc.dram_tensor("o", [BPC, OH, 128, NPIX], f32,
                         kind="ExternalOutput").ap()

    HEAD_ROWS = RPC + KH_EFF - 1            # x rows needed by first chunk: 14
    HEAD = HEAD_ROWS * WP                   # 924 cols

    # DMA descriptor issue costs ~0.6us on an engine queue; spread issues
    # over four otherwise-idle engine queues so they go out in parallel.
    _rr = [0]

    def dma(engines, dst, src):
        eng = engines[_rr[0] % len(engines)]
        _rr[0] += 1
        eng.dma_start(dst, src)

    with tile.TileContext(nc) as tc:
        with tc.tile_pool(name="xp", bufs=1) as xp, \
             tc.tile_pool(name="kp", bufs=1) as kp, \
             tc.tile_pool(name="bp", bufs=1) as bp, \
             tc.tile_pool(name="wu", bufs=1) as wu, \
             tc.tile_pool(name="ps", bufs=8, space="PSUM") as ps, \
             tc.tile_pool(name="op", bufs=4) as op:

            kt = kp.tile([CIN, OH * NTAPS * 128], mmdt, tag="k")
            bt = bp.tile([128, OH], f32, tag="bias")
            xts = [xp.tile([CIN, HP * WP], mmdt, tag=f"x{n}", name=f"x{n}")
                   for n in range(BPC)]

            # warmup tile for the PE clock (HAM) ramp: tiny zeros input,
            # DMA'd before everything so dummy matmuls start ~1us in
            wt = None
            if WARMUP:
                wt = wu.tile([128, 16], mmdt, tag="warm")
                nc.sync.dma_start(wt[:], wz_d[:])

            # --- input DMAs, priority-ordered, issued from 4 engines in
            # parallel, spread over the 16 HW queues ---
            ie = [nc.sync, nc.gpsimd, nc.scalar]
            # 1) first rows of image 0 (first matmul needs them + tap0 weights)
            for lo, hi in _splits(HEAD, 4):
                dma(ie, xts[0][:, lo:hi], x_d[0][:, lo:hi])
            # 2) weights for the first oh half, fine-grained so taps stream in
            for lo, hi in _splits(NTAPS * 128, 16):
                dma(ie, kt[:, lo:hi], k_d[:, lo:hi])
            # 3) rest of image 0
            for lo, hi in _splits(HP * WP - HEAD, 5):
                dma(ie, xts[0][:, HEAD + lo:HEAD + hi],
                    x_d[0][:, HEAD + lo:HEAD + hi])
            # 4) bias, second weight half, remaining images
            for h in range(OH):
                dma(ie, bt[:, h:h + 1], b_d[h])
            for lo, hi in _splits(NTAPS * 128, 8):
                off = NTAPS * 128
                dma(ie, kt[:, off + lo:off + hi], k_d[:, off + lo:off + hi])
            for n in range(1, BPC):
                for lo, hi in _splits(HP * WP, 6):
                    dma(ie, xts[n][:, lo:hi], x_d[n][:, lo:hi])

            # --- HAM warmup: dummy matmuls while inputs stream in ---
            for _ in range(WARMUP):
                pw = ps.tile([16, 16], f32, tag="acc")
                nc.tensor.matmul(pw[:], wt[:, 0:16], wt[:], start=True,
                                 stop=True)

            # --- the conv ---
            def do_group(n, h, c, xv):
                pt = ps.tile([128, CHUNK], f32, tag="acc")
                y0 = c * RPC
                for t, (kh, kw) in enumerate(
                        (kh, kw) for kh in range(KH_EFF)
                        for kw in range(K_W)):
                    rhs = xv[:, y0 + kh:y0 + kh + RPC, kw:kw + W]
                    off = ((h * KH_EFF + kh) * K_W + kw) * 128
                    nc.tensor.matmul(pt[:], kt[:, off:off + 128], rhs,
                                     start=(t == 0), stop=(t == NTAPS - 1))
                ot = op.tile([128, CHUNK], f32, tag="out")
                nc.scalar.activation(ot[:], pt[:],
                                     mybir.ActivationFunctionType.Identity,
                                     bias=bt[:, h:h + 1])
                # split the store so the flush of the last chunk isn't
                # bottlenecked on a single ~22GB/s DMA queue; the very last
                # store goes 8-way on the HW queues (SW queues drain slowly)
                last = (n == BPC - 1 and h == OH - 1 and c == NCHUNK - 1)
                oe = [nc.sync, nc.scalar] if last else [nc.sync, nc.gpsimd]
                for lo, hi in _splits(CHUNK, 8 if last else 2):
                    dma(oe, o_d[n, h][:, c * CHUNK + lo:c * CHUNK + hi],
                        ot[:, lo:hi])

            def do_block_tap_outer(n, h, xv):
                pts = [ps.tile([128, CHUNK], f32, tag="acc",
                               name=f"acc_{n}_{h}_{c}")
                       for c in range(NCHUNK)]
                for t, (kh, kw) in enumerate(
                        (kh, kw) for kh in range(KH_EFF)
                        for kw in range(K_W)):
                    off = ((h * KH_EFF + kh) * K_W + kw) * 128
                    for c in range(NCHUNK):
                        rhs = xv[:, c * RPC + kh:c * RPC + kh + RPC, kw:kw + W]
                        nc.tensor.matmul(pts[c][:], kt[:, off:off + 128], rhs,
                                         start=(t == 0),
                                         stop=(t == NTAPS - 1))
                for c in range(NCHUNK):
                    ot = op.tile([128, CHUNK], f32, tag="out")
                    nc.scalar.activation(ot[:], pts[c][:],
                                         mybir.ActivationFunctionType.Identity,
                                         bias=bt[:, h:h + 1])
                    last = (n == BPC - 1 and h == OH - 1 and c == NCHUNK - 1)
                    oe = [nc.sync, nc.gpsimd]
                    for lo, hi in _splits(CHUNK, 4 if last else 2):
                        dma(oe, o_d[n, h][:, c * CHUNK + lo:c * CHUNK + hi],
                            ot[:, lo:hi])

            for n in range(BPC):
                xv = xts[n][:].rearrange("p (h w) -> p h w", h=HP)
                for h in range(OH):
                    if ORDER == "tap":
                        do_block_tap_outer(n, h, xv)
                    else:
                        for c in range(NCHUNK):
                            do_group(n, h, c, xv)
    t0 = time.time()
    nc.compile()
    print(f"[kernel] bacc compile: {time.time()-t0:.1f}s", file=sys.stderr)
    return nc


def kernel(x: np.ndarray, weight: np.ndarray, bias: np.ndarray,
           P: np.ndarray) -> np.ndarray:
    global _NC_CACHE, _last_in_maps
    x = np.asarray(x, dtype=np.float32)
    weight = np.asarray(weight, dtype=np.float32)
    bias = np.asarray(bias, dtype=np.float32)
    P = np.asarray(P, dtype=np.float32)

    K = _build_kernel_np(weight, P[0])                    # (O, I, 7, 3)
    # device layout: [i, (oh, kh, kw, o128)]
    k_dev = np.ascontiguousarray(
        K.reshape(OH, 128, CIN, KH_EFF, K_W)
        .transpose(2, 0, 3, 4, 1)
        .reshape(CIN, OH * NTAPS * 128)).astype(_NP_DT)

    xpad = np.zeros((B, CIN, HP, WP), np.float32)
    xpad[:, :, PAD_H:PAD_H + H, PAD_W:PAD_W + W] = x
    xpad = xpad.reshape(B, CIN, HP * WP).astype(_NP_DT)

    b_dev = np.ascontiguousarray(bias.reshape(OH, 128, 1))

    if _NC_CACHE is None:
        t0 = time.time()
        _NC_CACHE = _build_bass()
        print(f"[kernel] build+compile total: {time.time()-t0:.1f}s",
              file=sys.stderr)

    wz = np.zeros((128, 16), _NP_DT)
    in_maps = [
        {"x": np.ascontiguousarray(xpad[i * BPC:(i + 1) * BPC]),
         "k": k_dev, "b": b_dev, **({"wz": wz} if WARMUP else {})}
        for i in range(N_CORES)
    ]
    _last_in_maps = in_maps
    t0 = time.time()
    res = bass_utils.run_bass_kernel_spmd(
        _NC_CACHE, in_maps, core_ids=list(range(N_CORES)))
    print(f"[kernel] run (incl. walrus compile on first call): "
          f"{time.time()-t0:.1f}s", file=sys.stderr)
    out = np.concatenate(
        [res.results[i]["o"].reshape(BPC, COUT, H, W)
         for i in range(N_CORES)], axis=0)
    return out
